# revision 11
# baseline (speedup 1.0000x reference)
"""Causal self-attention on 8 Trainium2 NeuronCores.

Sharding: B*H = 2*12 = 24 (batch, head) pairs -> 3 heads per core.
Core i handles batch i//4, heads 3*(i%4) .. 3*(i%4)+2.
Each core computes q/k/v projections for its 3 heads (tensor-parallel slice
of wq/wk/wv), causal attention, and a partial out-projection against its
192 columns of wo. Host sums the 4 partials per batch (the "all-reduce").

v4:
  - Projection (phase A) FUSED into the attention loop: the T-block tb's
    x-load/transpose/projection/v-transpose work is emitted in small
    chunks spread across the attention stages of q-block tb-1, filling
    the PE slack of the ACT-bound attention cadence (and the early
    q-blocks, where exp work is scarce, absorb most of it).
  - Software-pipelined attention (S matmuls one stage ahead of PV;
    division epilogue and out-proj deferred one/two stages) so the
    in-order PE queue never stalls behind the ACT exp path, keeping the
    HAM clock gate at 2.4 GHz.
  - 512-wide q blocks: half the S/PV instruction count of 256-wide.
  - Causal masking via gpsimd affine_select (Pool engine, otherwise idle)
    on the four diagonal blocks of each unit.
  - Division epilogue reads the PV accumulator directly from PSUM
    (reciprocal of the denominator row, ones-matmul partition broadcast,
    tensor_mul) - no accumulator copy.
  - All short-lived 2KB/partition PSUM tiles (transpose groups, proj,
    bc broadcast, out-proj, v-transpose) share one pool tag so the whole
    kernel fits the 8 PSUM banks: S double-buffer 4 + acc 1 + shared 3.

Per-core kernel (all fp32 data; matmuls run as float32r = full-rate fp32):
  - x [T, 768] loaded in natural layout, PE-transposed to xT tiles.
  - qT/kT computed in [64, T] layout; v computed via vT then PE-transposed
    to natural [T, 64] with a ones column appended (softmax denominators).
  - S_T[kblock, qblock] = K_blk @ Q_blk.T  (contraction d=64)
  - P_T = exp(S_T / 8)  on ACT over [128, 1024] groups (2 kblocks)
  - attnU_T[65, TQ] += Vaug_blk.T @ P_T  (row 64 = softmax denominator)
  - y[T,768] partial = attnT.T @ woT_slice.
No max-subtraction in softmax: logits here have |.| <~ 2, exp is safe.

Partition-base alignment: per-head pairs live at the same partition offset:
  q01 [128,T] = qT_h0 (rows 0:64) | qT_h1 (rows 64:128)
  k01 [128,T] = kT_h0 | kT_h1
  qv0 [128,T] = qT_h2 | vT_h0
  kv1 [128,T] = kT_h2 | vT_h1
  v2t [64,T]  = vT_h2
"""

import numpy as np

import concourse.bass as bass
import concourse.mybir as mybir
from concourse import bacc
from concourse import tile
from concourse.bass_utils import run_bass_kernel_spmd
from concourse.masks import make_identity

F32 = mybir.dt.float32
F32R = mybir.dt.float32r

EMBED = 768
NHEAD = 12
DH = 64
B = 2
T = 4096
HPC = 3          # heads per core
CH = HPC * DH    # 192 channels per core
NCORES = 8
QW = 512         # q-block width == projection T-block width


def build_program(t=T):
    """Build the single-core SPMD Bass program."""
    nqb = t // QW

    nc = bacc.Bacc("TRN2", target_bir_lowering=False, debug=False,
                   num_devices=NCORES)

    x_d = nc.dram_tensor("x", [t, EMBED], F32, kind="ExternalInput")
    # columns: q0,q1 | k0,k1 | q2,v0 | k2,v1 | v2   (64 each)
    wqkv_d = nc.dram_tensor("wqkvT", [EMBED, 576], F32, kind="ExternalInput")
    bqkv_d = nc.dram_tensor("bqkv", [576, 1], F32, kind="ExternalInput")
    wo_d = nc.dram_tensor("woT", [CH, EMBED], F32, kind="ExternalInput")
    y_d = nc.dram_tensor("y", [t, EMBED], F32, kind="ExternalOutput")

    Act = mybir.ActivationFunctionType

    with tile.TileContext(nc) as tc:
        with (
            tc.tile_pool(name="const", bufs=1) as cpool,
            tc.tile_pool(name="persist", bufs=1) as perm,
        ):
            ident = cpool.tile([128, 128], F32, tag="ident")
            make_identity(nc, ident)
            identr = cpool.tile([128, 128], F32R, tag="identr")
            nc.vector.tensor_copy(identr, ident)
            ones_t = cpool.tile([128, 64], F32R, tag="ones")
            nc.gpsimd.memset(ones_t.bitcast(F32), 1.0)

            # weights: raw tiles in a temporary pool; rounded fp32r persists
            wqkv_sb = [cpool.tile([128, 576], F32R, name=f"wqkv{kt}",
                                  tag=f"wqkv{kt}") for kt in range(6)]
            wo_sb = [cpool.tile([64, EMBED], F32R, name=f"wo{h}",
                                tag=f"wo{h}") for h in range(3)]
            bias_sb = []
            for mc in range(5):
                mw = 128 if mc < 4 else 64
                b_t = cpool.tile([128, 1], F32, name=f"bias{mc}",
                                 tag=f"bias{mc}")
                nc.sync.dma_start(b_t[:mw, :],
                                  bqkv_d[mc * 128:mc * 128 + mw, :])
                bias_sb.append(b_t)
            with tc.tile_pool(name="wraw", bufs=1) as wraw:
                for kt in range(6):
                    w_raw = wraw.tile([128, 576], F32, name=f"wqkvraw{kt}",
                                      tag=f"wqkvraw{kt}")
                    nc.sync.dma_start(w_raw,
                                      wqkv_d[kt * 128:(kt + 1) * 128, :])
                    nc.vector.tensor_copy(wqkv_sb[kt], w_raw)
                for h in range(3):
                    wo_raw = wraw.tile([64, EMBED], F32, name=f"woraw{h}",
                                       tag=f"woraw{h}")
                    nc.sync.dma_start(wo_raw, wo_d[h * 64:(h + 1) * 64, :])
                    nc.vector.tensor_copy(wo_sb[h], wo_raw)

            # persistent activations
            q01 = perm.tile([128, t], F32R, tag="q01")
            k01 = perm.tile([128, t], F32R, tag="k01")
            qv0 = perm.tile([128, t], F32R, tag="qv0")
            kv1 = perm.tile([128, t], F32R, tag="kv1")
            v2t = perm.tile([64, t], F32R, tag="v2t")
            vs = [perm.tile([128, (t // 128) * 65], F32R, name=f"vs{h}",
                            tag=f"vs{h}")
                  for h in range(3)]
            for h in range(3):
                nc.gpsimd.memset(vs[h].bitcast(F32), 1.0)

            proj_dest = [q01, k01, qv0, kv1, v2t]

            def q_ap(h):
                return (q01[0:64], q01[64:128], qv0[0:64])[h]

            def k_ap(h):
                return (k01[0:64], k01[64:128], kv1[0:64])[h]

            v_src = [qv0[64:128], kv1[64:128], v2t[0:64]]
            v_idn = [identr[64:128, 64:128], identr[64:128, 64:128],
                     identr[0:64, 0:64]]

            # ---------- projection work for T-block tb, as chunks ----------
            def a_chunks(tb):
                """Closures emitting T-block tb's projection work in small
                PE bursts. Order matters: loads -> transposes -> proj ->
                v transpose."""
                xns = []
                xts = []

                def c_load():
                    for i in range(4):
                        row0 = tb * QW + i * 128
                        xn = xpool.tile([128, EMBED], F32, tag="xn",
                                        name=f"xn{tb}_{i}")
                        nc.sync.dma_start(xn, x_d[row0:row0 + 128, :])
                        xns.append(xn)
                    for ct in range(6):
                        xts.append(xtpool.tile(
                            [128, 512], F32R, tag=f"xt{ct}",
                            name=f"xt{ct}_{tb}"))

                def c_tr(ct):
                    def f():
                        tpg = upsum.tile([128, 512], F32, tag="u2k",
                                         name=f"tpg{tb}_{ct}")
                        for i in range(4):
                            nc.tensor.transpose(
                                tpg[:, i * 128:(i + 1) * 128],
                                xns[i][:, ct * 128:(ct + 1) * 128], ident)
                        if ct < 3:
                            nc.scalar.copy(xts[ct], tpg)
                        else:
                            nc.vector.tensor_copy(xts[ct], tpg)
                    return f

                def c_proj(mc):
                    def f():
                        mw = 128 if mc < 4 else 64
                        ps = upsum.tile([128, 512], F32, tag="u2k",
                                        name=f"proj{tb}_{mc}")
                        for ct in range(6):
                            nc.tensor.matmul(
                                ps[:mw, :],
                                lhsT=wqkv_sb[ct][:, mc * 128:mc * 128 + mw],
                                rhs=xts[ct],
                                start=(ct == 0), stop=(ct == 5))
                        dest = proj_dest[mc][:, tb * QW:(tb + 1) * QW]
                        nc.scalar.activation(dest, ps[:mw, :], Act.Identity,
                                             bias=bias_sb[mc][:mw, :],
                                             scale=1.0)
                    return f

                def c_vt(h):
                    def f():
                        vtile = upsum.tile([128, 512], F32R, tag="u2k",
                                           name=f"vt{h}_{tb}")
                        for i in range(4):
                            ck = tb * 4 + i
                            nc.tensor.transpose(
                                vtile[:, i * 64:(i + 1) * 64],
                                v_src[h][:, ck * 128:(ck + 1) * 128],
                                v_idn[h])
                        src = vtile[:, 0:256].rearrange(
                            "p (c w) -> p c w", w=64)
                        dst = vs[h].rearrange("p (c w) -> p c w", w=65)[
                            :, tb * 4:tb * 4 + 4, 0:64]
                        nc.vector.tensor_copy(dst, src)
                    return f

                chunks = [c_load]
                chunks += [c_tr(ct) for ct in range(6)]
                chunks += [c_proj(mc) for mc in range(5)]
                chunks += [c_vt(h) for h in range(3)]
                return chunks

            # ---------------- attention stages -----------------------------
            stages = []
            for qb in range(nqb):
                kbn = (qb + 1) * QW // 128
                ng = kbn // 2
                for h in range(3):
                    for g in range(ng):
                        stages.append((qb, h, g, 2 * g, g == ng - 1))
            nstages = len(stages)

            sp_t = {}
            pt_t = {}
            acc_t = {}
            rec_t = {}
            attn = {}
            deferred = {}

            def defer(slot, fn):
                deferred.setdefault(slot, []).append(fn)

            def emit_S(i):
                qb, h, g, kb0, last = stages[i]
                sp = spsum.tile([128, 2 * QW], F32, tag="s",
                                name=f"s{qb}_{h}_{g}")
                sp_t[i] = sp
                q_sl = slice(qb * QW, (qb + 1) * QW)
                for j in range(2):
                    kbi = kb0 + j
                    nc.tensor.matmul(
                        sp[:, j * QW:(j + 1) * QW],
                        lhsT=k_ap(h)[:, kbi * 128:(kbi + 1) * 128],
                        rhs=q_ap(h)[:, q_sl],
                        start=True, stop=True)

            def emit_exp_mask(i):
                qb, h, g, kb0, last = stages[i]
                kbn = (qb + 1) * QW // 128
                pt = ppool.tile([128, 2 * QW], F32R, tag="p",
                                name=f"p{qb}_{h}_{g}")
                pt_t[i] = pt
                nc.scalar.activation(pt, sp_t[i], Act.Exp,
                                     bias=0.0, scale=0.125)
                for j in range(2):
                    kbi = kb0 + j
                    if kbi >= kbn - 4:
                        v = pt[:, j * QW:(j + 1) * QW]
                        nc.gpsimd.affine_select(
                            out=v, in_=v,
                            compare_op=mybir.AluOpType.is_ge,
                            fill=0.0, base=qb * QW - kbi * 128,
                            pattern=[[1, QW]], channel_multiplier=-1)

            def emit_PV(i):
                qb, h, g, kb0, last = stages[i]
                if g == 0:
                    acc_t[(qb, h)] = accpsum.tile(
                        [65, QW], F32, tag="acc", name=f"acc{qb}_{h}")
                acc = acc_t[(qb, h)]
                kbn = (qb + 1) * QW // 128
                pt = pt_t.pop(i)
                for j in range(2):
                    kbi = kb0 + j
                    nc.tensor.matmul(
                        acc,
                        lhsT=vs[h][:, kbi * 65:kbi * 65 + 65],
                        rhs=pt[:, j * QW:(j + 1) * QW],
                        start=(kbi == 0), stop=(kbi == kbn - 1))
                sp_t.pop(i)

            def emit_recip(qb, h):
                acc = acc_t[(qb, h)]
                rec = rpool.tile([65, QW], F32R, tag="rec",
                                 name=f"rec{qb}_{h}")
                rec_t[(qb, h)] = rec
                with nc.allow_low_precision(reason="fp32r rounding"):
                    nc.vector.reciprocal(rec[64:65], acc[64:65])

            def emit_div(qb, h):
                acc = acc_t.pop((qb, h))
                rec = rec_t.pop((qb, h))
                bc = upsum.tile([128, 512], F32, tag="u2k",
                                name=f"bc{qb}_{h}")
                nc.tensor.matmul(bc[0:64, :], lhsT=ones_t[64:65, :],
                                 rhs=rec[64:65, :],
                                 start=True, stop=True)
                if h == 0:
                    attn[qb] = [apool.tile([64, QW], F32R, tag=f"attn{hh}",
                                           name=f"attn{hh}_{qb}")
                                for hh in range(3)]
                nc.vector.tensor_mul(attn[qb][h], acc[0:64, :], bc[0:64, :])

            def emit_outproj(qb, mt):
                at = attn[qb]
                t_sl = slice(mt * 128, (mt + 1) * 128)
                row0 = qb * QW + mt * 128
                ys = ysb.tile([128, EMBED], F32, tag="ys",
                              name=f"ys{qb}_{mt}")
                for nh in range(2):
                    n_sl = slice(nh * 384, (nh + 1) * 384)
                    yp = upsum.tile([128, 512], F32, tag="u2k",
                                    name=f"y{qb}_{mt}_{nh}")
                    for h in range(3):
                        nc.tensor.matmul(yp[:, 0:384], lhsT=at[h][:, t_sl],
                                         rhs=wo_sb[h][:, n_sl],
                                         start=(h == 0), stop=(h == 2))
                    nc.vector.tensor_copy(ys[:, n_sl], yp[:, 0:384])
                nc.sync.dma_start(y_d[row0:row0 + 128, :], ys)
                if mt == 3:
                    attn.pop(qb)

            # ---------------- fused emission loop --------------------------
            pending_a = a_chunks(0)
            while pending_a:
                pending_a.pop(0)()
            pending_a = a_chunks(1)

            emit_S(0)
            for i in range(nstages):
                qb, h, g, kb0, last = stages[i]
                if i + 1 < nstages:
                    if stages[i + 1][0] != qb:
                        # q-block boundary: next block's projections must be
                        # fully emitted before its S matmuls
                        while pending_a:
                            pending_a.pop(0)()
                        if qb + 2 < nqb:
                            pending_a = a_chunks(qb + 2)
                        else:
                            pending_a = []
                    emit_S(i + 1)
                emit_exp_mask(i)
                # deferred actions run BEFORE emit_PV: the division of the
                # previous unit must be emitted before this unit's first PV
                # rotates onto the single acc buffer
                for fn in deferred.pop(i, ()):
                    fn()
                emit_PV(i)
                if last:
                    emit_recip(qb, h)
                # spread next T-block's projection work into this q-block
                for _ in range(2):
                    if pending_a:
                        pending_a.pop(0)()
                if last:
                    defer(i + 1, lambda qb=qb, h=h: emit_div(qb, h))
                    if h == 2:
                        for mt in range(4):
                            defer(i + 2 + mt,
                                  lambda qb=qb, mt=mt: emit_outproj(qb, mt))
            for slot in sorted(deferred):
                for fn in deferred[slot]:
                    fn()
    nc.compile()
    return nc


_PROG_CACHE = {}


def _get_program(t=T):
    if t not in _PROG_CACHE:
        _PROG_CACHE[t] = build_program(t)
    return _PROG_CACHE[t]


def make_in_maps(x, wq, bq, wk, bk, wv, bv, wo):
    in_maps = []
    for core in range(NCORES):
        b = core // 4
        hs = (core % 4) * HPC
        sl = [slice((hs + h) * DH, (hs + h + 1) * DH) for h in range(HPC)]
        # columns: q0,q1 | k0,k1 | q2,v0 | k2,v1 | v2
        cols = [wq[sl[0]].T, wq[sl[1]].T, wk[sl[0]].T, wk[sl[1]].T,
                wq[sl[2]].T, wv[sl[0]].T, wk[sl[2]].T, wv[sl[1]].T,
                wv[sl[2]].T]
        biases = [bq[sl[0]], bq[sl[1]], bk[sl[0]], bk[sl[1]],
                  bq[sl[2]], bv[sl[0]], bk[sl[2]], bv[sl[1]], bv[sl[2]]]
        wqkvT = np.ascontiguousarray(np.concatenate(cols, axis=1),
                                     dtype=np.float32)
        bqkv = np.ascontiguousarray(
            np.concatenate(biases)[:, None], dtype=np.float32)
        ch = slice(hs * DH, (hs + HPC) * DH)
        woT = np.ascontiguousarray(wo[:, ch].T, dtype=np.float32)
        in_maps.append({
            "x": np.ascontiguousarray(x[b], dtype=np.float32),
            "wqkvT": wqkvT,
            "bqkv": bqkv,
            "woT": woT,
        })
    return in_maps


def run(inputs, t=T, trace=False, **kw):
    """Run on hardware; returns (y, BassKernelResults)."""
    arrs = {k: np.asarray(v, dtype=np.float32) for k, v in inputs.items()}
    nc = _get_program(t)
    in_maps = make_in_maps(**arrs)
    res = run_bass_kernel_spmd(nc, in_maps, list(range(NCORES)),
                               trace=trace, **kw)
    outs = [np.asarray(m["y"], dtype=np.float32) for m in res.results]
    y = np.empty((B, t, EMBED), dtype=np.float32)
    for b in range(B):
        y[b] = outs[4 * b] + outs[4 * b + 1] + outs[4 * b + 2] + outs[4 * b + 3]
    return y, res


def kernel(**inputs):
    y, _ = run(inputs)
    return y


# revision 18
# speedup vs baseline: 1.1461x; 1.1461x over previous
"""Causal self-attention on 8 Trainium2 NeuronCores.

Sharding: B*H = 2*12 = 24 (batch, head) pairs -> 3 heads per core.
Core i handles batch i//4, heads 3*(i%4) .. 3*(i%4)+2.
Each core computes q/k/v projections for its 3 heads (tensor-parallel slice
of wq/wk/wv), causal attention, and a partial out-projection against its
192 columns of wo. Host sums the 4 partials per batch (the "all-reduce").

v3:
  - Software-pipelined attention loop (S matmuls one stage ahead of PV;
    epilogues and out-proj deferred one/two stages) so the in-order PE
    queue never stalls behind the ACT exp path, keeping the HAM clock
    gate at 2.4 GHz.
  - 512-wide q blocks: same FLOPs, half the S/PV instruction count of
    256-wide blocks (instruction/semaphore overhead is significant on HW).
  - Causal masking via gpsimd affine_select on the four diagonal blocks
    of each unit (Pool engine, otherwise idle).
  - Phase A transposes grouped per weight-column block into [128,512]
    PSUM tiles -> single PSUM->SBUF copy each, split between ACT and DVE.
  - v transposed in fp32r (1.5 cyc/row) and copied into the augmented
    v tile with one strided copy per (head, T-block).

Per-core kernel (all fp32 data; matmuls run as float32r = full-rate fp32):
  - x [T, 768] loaded in natural layout, PE-transposed to xT tiles.
  - qT/kT computed in [64, T] layout; v computed via vT then PE-transposed
    to natural [T, 64] with a ones column appended (softmax denominators).
  - S_T[kblock, qblock] = K_blk @ Q_blk.T  (contraction d=64)
  - P_T = exp(S_T / 8)  on ACT over [128, 1024] groups (2 kblocks)
  - attnU_T[65, TQ] += Vaug_blk.T @ P_T  (row 64 = softmax denominator)
  - divide via reciprocal + partition-broadcast matmul, then
    y[T,768] partial = attnT.T @ woT_slice.
No max-subtraction in softmax: logits here have |.| <~ 2, exp is safe.

Partition-base alignment: per-head pairs live at the same partition offset:
  q01 [128,T] = qT_h0 (rows 0:64) | qT_h1 (rows 64:128)
  k01 [128,T] = kT_h0 | kT_h1
  qv0 [128,T] = qT_h2 | vT_h0
  kv1 [128,T] = kT_h2 | vT_h1
  v2t [64,T]  = vT_h2
"""

import numpy as np

import concourse.bass as bass
import concourse.mybir as mybir
from concourse import bacc
from concourse import tile
from concourse.bass_utils import run_bass_kernel_spmd
from concourse.masks import make_identity

F32 = mybir.dt.float32
F32R = mybir.dt.float32r

EMBED = 768
NHEAD = 12
DH = 64
B = 2
T = 4096
HPC = 3          # heads per core
CH = HPC * DH    # 192 channels per core
NCORES = 8
QW = 512         # q-block width in phase C


def build_program(t=T):
    """Build the single-core SPMD Bass program."""
    nqb = t // QW    # q blocks
    ntb = t // 512   # projection T-blocks of 512

    nc = bacc.Bacc("TRN2", target_bir_lowering=False, debug=False,
                   num_devices=NCORES)

    x_d = nc.dram_tensor("x", [t, EMBED], F32, kind="ExternalInput")
    # columns: q0,q1 | k0,k1 | q2,v0 | k2,v1 | v2   (64 each)
    wqkv_d = nc.dram_tensor("wqkvT", [EMBED, 576], F32, kind="ExternalInput")
    bqkv_d = nc.dram_tensor("bqkv", [576, 1], F32, kind="ExternalInput")
    wo_d = nc.dram_tensor("woT", [CH, EMBED], F32, kind="ExternalInput")
    y_d = nc.dram_tensor("y", [t, EMBED], F32, kind="ExternalOutput")

    Act = mybir.ActivationFunctionType

    with tile.TileContext(nc) as tc:
        with (
            tc.tile_pool(name="const", bufs=1) as cpool,
            tc.tile_pool(name="persist", bufs=1) as perm,
        ):
            ident = cpool.tile([128, 128], F32, tag="ident")
            make_identity(nc, ident)
            identr = cpool.tile([128, 128], F32R, tag="identr")
            nc.vector.tensor_copy(identr, ident)
            ones_t = cpool.tile([128, 64], F32R, tag="ones")
            nc.gpsimd.memset(ones_t.bitcast(F32), 1.0)

            # weights: raw tiles live in a temporary pool, converted
            # (rounded) fp32r copies persist
            wqkv_sb = [cpool.tile([128, 576], F32R, name=f"wqkv{kt}",
                                  tag=f"wqkv{kt}") for kt in range(6)]
            wo_sb = [cpool.tile([64, EMBED], F32R, name=f"wo{h}",
                                tag=f"wo{h}") for h in range(3)]
            bias_sb = []
            for mc in range(5):
                mw = 128 if mc < 4 else 64
                b_t = cpool.tile([128, 1], F32, name=f"bias{mc}",
                                 tag=f"bias{mc}")
                nc.sync.dma_start(b_t[:mw, :],
                                  bqkv_d[mc * 128:mc * 128 + mw, :])
                bias_sb.append(b_t)
            with tc.tile_pool(name="wraw", bufs=1) as wraw:
                for kt in range(6):
                    w_raw = wraw.tile([128, 576], F32, name=f"wqkvraw{kt}",
                                      tag=f"wqkvraw{kt}")
                    nc.sync.dma_start(w_raw,
                                      wqkv_d[kt * 128:(kt + 1) * 128, :])
                    nc.vector.tensor_copy(wqkv_sb[kt], w_raw)
                for h in range(3):
                    wo_raw = wraw.tile([64, EMBED], F32, name=f"woraw{h}",
                                       tag=f"woraw{h}")
                    nc.sync.dma_start(wo_raw, wo_d[h * 64:(h + 1) * 64, :])
                    nc.vector.tensor_copy(wo_sb[h], wo_raw)

            # persistent activations
            q01 = perm.tile([128, t], F32R, tag="q01")
            k01 = perm.tile([128, t], F32R, tag="k01")
            qv0 = perm.tile([128, t], F32R, tag="qv0")
            kv1 = perm.tile([128, t], F32R, tag="kv1")
            v2t = perm.tile([64, t], F32R, tag="v2t")
            # v natural, 65-wide per 128-row chunk (col 64 = ones)
            vs = [perm.tile([128, (t // 128) * 65], F32R, name=f"vs{h}",
                            tag=f"vs{h}")
                  for h in range(3)]
            for h in range(3):
                nc.gpsimd.memset(vs[h].bitcast(F32), 1.0)

            proj_dest = [q01, k01, qv0, kv1, v2t]

            def q_ap(h):
                return (q01[0:64], q01[64:128], qv0[0:64])[h]

            def k_ap(h):
                return (k01[0:64], k01[64:128], kv1[0:64])[h]

            # ---------------- phase A: projections (+ v transpose) --------
            v_src = [qv0[64:128], kv1[64:128], v2t[0:64]]
            v_idn = [identr[64:128, 64:128], identr[64:128, 64:128],
                     identr[0:64, 0:64]]
            with (
                tc.tile_pool(name="xpool", bufs=4) as xpool,
                tc.tile_pool(name="xtpool", bufs=2) as xtpool,
                tc.tile_pool(name="tpsum", bufs=3, space="PSUM") as tpsum,
                tc.tile_pool(name="vpsum", bufs=2, space="PSUM") as vpsum,
                tc.tile_pool(name="projpsum", bufs=2, space="PSUM") as projpsum,
            ):
                for tb in range(ntb):
                    xns = []
                    for i in range(4):
                        row0 = tb * 512 + i * 128
                        xn = xpool.tile([128, EMBED], F32, tag="xn",
                                        name=f"xn{tb}_{i}")
                        nc.sync.dma_start(xn, x_d[row0:row0 + 128, :])
                        xns.append(xn)
                    xts = [xtpool.tile([128, 512], F32R, tag=f"xt{ct}",
                                       name=f"xt{ct}_{tb}")
                           for ct in range(6)]
                    for ct in range(6):
                        tpg = tpsum.tile([128, 512], F32, tag="tpg",
                                         name=f"tpg{tb}_{ct}")
                        for i in range(4):
                            nc.tensor.transpose(
                                tpg[:, i * 128:(i + 1) * 128],
                                xns[i][:, ct * 128:(ct + 1) * 128], ident)
                        if ct < 3:
                            nc.scalar.copy(xts[ct], tpg)
                        else:
                            nc.vector.tensor_copy(xts[ct], tpg)
                    for mc in range(5):
                        mw = 128 if mc < 4 else 64
                        ps = projpsum.tile([mw, 512], F32, tag="proj",
                                           name=f"proj{tb}_{mc}")
                        for ct in range(6):
                            nc.tensor.matmul(
                                ps,
                                lhsT=wqkv_sb[ct][:, mc * 128:mc * 128 + mw],
                                rhs=xts[ct],
                                start=(ct == 0), stop=(ct == 5))
                        dest = proj_dest[mc][:, tb * 512:(tb + 1) * 512]
                        nc.scalar.activation(dest, ps, Act.Identity,
                                             bias=bias_sb[mc][:mw, :],
                                             scale=1.0)
                    # v transpose for this T-block (4 column chunks of 128)
                    for h in range(3):
                        vtile = vpsum.tile([128, 256], F32R, tag="vt",
                                           name=f"vt{h}_{tb}")
                        for i in range(4):
                            ck = tb * 4 + i
                            nc.tensor.transpose(
                                vtile[:, i * 64:(i + 1) * 64],
                                v_src[h][:, ck * 128:(ck + 1) * 128],
                                v_idn[h])
                        src = vtile.rearrange("p (c w) -> p c w", w=64)
                        dst = vs[h].rearrange("p (c w) -> p c w", w=65)[
                            :, tb * 4:tb * 4 + 4, 0:64]
                        nc.vector.tensor_copy(dst, src)

            # ---------------- phase C/D: attention + out-proj -------------
            # stage = 2 kblocks of [128, QW]; unit = (qb, h)
            stages = []
            for qb in range(nqb):
                kbn = (qb + 1) * QW // 128
                ng = kbn // 2
                for h in range(3):
                    for g in range(ng):
                        stages.append((qb, h, g, 2 * g, g == ng - 1))
            nstages = len(stages)

            with (
                tc.tile_pool(name="spsum", bufs=2, space="PSUM") as spsum,
                tc.tile_pool(name="accpsum", bufs=1, space="PSUM") as accpsum,
                tc.tile_pool(name="bcpsum", bufs=1, space="PSUM") as bcpsum,
                tc.tile_pool(name="ypsum", bufs=2, space="PSUM") as ypsum,
                tc.tile_pool(name="ppool", bufs=3) as ppool,
                tc.tile_pool(name="apool", bufs=2) as apool,
                tc.tile_pool(name="rpool", bufs=2) as rpool,
                tc.tile_pool(name="ysb", bufs=2) as ysb,
            ):
                sp_t = {}    # stage idx -> S psum tile
                pt_t = {}    # stage idx -> P sbuf tile
                acc_t = {}   # (qb, h) -> acc psum tile [65, QW]
                sb_t = {}    # (qb, h) -> accsb sbuf tile
                attn = {}    # qb -> [3] attn tiles [64, QW]
                deferred = {}  # slot idx -> list of closures

                def defer(slot, fn):
                    deferred.setdefault(slot, []).append(fn)

                def emit_S(i):
                    qb, h, g, kb0, last = stages[i]
                    sp = spsum.tile([128, 2 * QW], F32, tag="s",
                                    name=f"s{qb}_{h}_{g}")
                    sp_t[i] = sp
                    q_sl = slice(qb * QW, (qb + 1) * QW)
                    for j in range(2):
                        kbi = kb0 + j
                        nc.tensor.matmul(
                            sp[:, j * QW:(j + 1) * QW],
                            lhsT=k_ap(h)[:, kbi * 128:(kbi + 1) * 128],
                            rhs=q_ap(h)[:, q_sl],
                            start=True, stop=True)

                def emit_exp_mask(i):
                    qb, h, g, kb0, last = stages[i]
                    kbn = (qb + 1) * QW // 128
                    pt = ppool.tile([128, 2 * QW], F32R, tag="p",
                                    name=f"p{qb}_{h}_{g}")
                    pt_t[i] = pt
                    nc.scalar.activation(pt, sp_t[i], Act.Exp,
                                         bias=0.0, scale=0.125)
                    # diagonal kblocks (kbi >= kbn-4):
                    # keep iff (qb*QW + u) - (kbi*128 + si) >= 0
                    for j in range(2):
                        kbi = kb0 + j
                        if kbi >= kbn - 4:
                            v = pt[:, j * QW:(j + 1) * QW]
                            nc.gpsimd.affine_select(
                                out=v, in_=v,
                                compare_op=mybir.AluOpType.is_ge,
                                fill=0.0, base=qb * QW - kbi * 128,
                                pattern=[[1, QW]], channel_multiplier=-1)

                def emit_PV(i):
                    qb, h, g, kb0, last = stages[i]
                    if g == 0:
                        acc_t[(qb, h)] = accpsum.tile(
                            [65, QW], F32, tag="acc", name=f"acc{qb}_{h}")
                    acc = acc_t[(qb, h)]
                    kbn = (qb + 1) * QW // 128
                    pt = pt_t.pop(i)
                    for j in range(2):
                        kbi = kb0 + j
                        nc.tensor.matmul(
                            acc,
                            lhsT=vs[h][:, kbi * 65:kbi * 65 + 65],
                            rhs=pt[:, j * QW:(j + 1) * QW],
                            start=(kbi == 0), stop=(kbi == kbn - 1))
                    sp_t.pop(i)

                def emit_epi1(qb, h):
                    # copy acc out of PSUM right away (frees the bank for
                    # the next head's accumulation)
                    accT = acc_t.pop((qb, h))
                    accsb = rpool.tile([65, QW], F32, tag="accsb",
                                       name=f"accsb{qb}_{h}")
                    nc.vector.tensor_copy(accsb, accT)
                    sb_t[(qb, h)] = accsb

                def emit_epi2(qb, h):
                    accsb = sb_t.pop((qb, h))
                    rec = rpool.tile([65, QW], F32R, tag="rec",
                                     name=f"rec{qb}_{h}")
                    with nc.allow_low_precision(
                            reason="fp32r operand rounding"):
                        nc.vector.reciprocal(rec[64:65], accsb[64:65])
                    bc = bcpsum.tile([64, QW], F32, tag="bc",
                                     name=f"bc{qb}_{h}")
                    nc.tensor.matmul(bc, lhsT=ones_t[64:65, :],
                                     rhs=rec[64:65, :],
                                     start=True, stop=True)
                    if h == 0:
                        attn[qb] = [apool.tile([64, QW], F32R,
                                               tag=f"attn{hh}",
                                               name=f"attn{hh}_{qb}")
                                    for hh in range(3)]
                    nc.vector.tensor_mul(attn[qb][h], accsb[0:64], bc)

                def emit_outproj(qb, mt):
                    at = attn[qb]
                    t_sl = slice(mt * 128, (mt + 1) * 128)
                    row0 = qb * QW + mt * 128
                    ys = ysb.tile([128, EMBED], F32, tag="ys",
                                  name=f"ys{qb}_{mt}")
                    for nh in range(2):
                        n_sl = slice(nh * 384, (nh + 1) * 384)
                        yp = ypsum.tile([128, 384], F32, tag="y",
                                        name=f"y{qb}_{mt}_{nh}")
                        for h in range(3):
                            nc.tensor.matmul(yp, lhsT=at[h][:, t_sl],
                                             rhs=wo_sb[h][:, n_sl],
                                             start=(h == 0), stop=(h == 2))
                        nc.vector.tensor_copy(ys[:, n_sl], yp)
                    nc.sync.dma_start(y_d[row0:row0 + 128, :], ys)
                    if mt == 3:
                        attn.pop(qb)

                emit_S(0)
                for i in range(nstages):
                    qb, h, g, kb0, last = stages[i]
                    if i + 1 < nstages:
                        emit_S(i + 1)
                    emit_exp_mask(i)
                    emit_PV(i)
                    if last:
                        emit_epi1(qb, h)
                    for fn in deferred.pop(i, ()):
                        fn()
                    if last:
                        defer(i + 1, lambda qb=qb, h=h: emit_epi2(qb, h))
                        if h == 2:
                            # spread the 4 out-proj row blocks over the
                            # next stages
                            for mt in range(4):
                                defer(i + 2 + mt,
                                      lambda qb=qb, mt=mt:
                                      emit_outproj(qb, mt))
                for slot in sorted(deferred):
                    for fn in deferred[slot]:
                        fn()
    nc.compile()
    return nc


_PROG_CACHE = {}


def _get_program(t=T):
    if t not in _PROG_CACHE:
        _PROG_CACHE[t] = build_program(t)
    return _PROG_CACHE[t]


def make_in_maps(x, wq, bq, wk, bk, wv, bv, wo):
    in_maps = []
    for core in range(NCORES):
        b = core // 4
        hs = (core % 4) * HPC
        sl = [slice((hs + h) * DH, (hs + h + 1) * DH) for h in range(HPC)]
        # columns: q0,q1 | k0,k1 | q2,v0 | k2,v1 | v2
        cols = [wq[sl[0]].T, wq[sl[1]].T, wk[sl[0]].T, wk[sl[1]].T,
                wq[sl[2]].T, wv[sl[0]].T, wk[sl[2]].T, wv[sl[1]].T,
                wv[sl[2]].T]
        biases = [bq[sl[0]], bq[sl[1]], bk[sl[0]], bk[sl[1]],
                  bq[sl[2]], bv[sl[0]], bk[sl[2]], bv[sl[1]], bv[sl[2]]]
        wqkvT = np.ascontiguousarray(np.concatenate(cols, axis=1),
                                     dtype=np.float32)
        bqkv = np.ascontiguousarray(
            np.concatenate(biases)[:, None], dtype=np.float32)
        ch = slice(hs * DH, (hs + HPC) * DH)
        woT = np.ascontiguousarray(wo[:, ch].T, dtype=np.float32)
        in_maps.append({
            "x": np.ascontiguousarray(x[b], dtype=np.float32),
            "wqkvT": wqkvT,
            "bqkv": bqkv,
            "woT": woT,
        })
    return in_maps


def run(inputs, t=T, trace=False, **kw):
    """Run on hardware; returns (y, BassKernelResults)."""
    arrs = {k: np.asarray(v, dtype=np.float32) for k, v in inputs.items()}
    nc = _get_program(t)
    in_maps = make_in_maps(**arrs)
    res = run_bass_kernel_spmd(nc, in_maps, list(range(NCORES)),
                               trace=trace, **kw)
    outs = [np.asarray(m["y"], dtype=np.float32) for m in res.results]
    y = np.empty((B, t, EMBED), dtype=np.float32)
    for b in range(B):
        y[b] = outs[4 * b] + outs[4 * b + 1] + outs[4 * b + 2] + outs[4 * b + 3]
    return y, res


def kernel(**inputs):
    y, _ = run(inputs)
    return y


# revision 19
# speedup vs baseline: 1.3893x; 1.2122x over previous
"""Causal self-attention on 8 Trainium2 NeuronCores — v5 (fused phases).

See kernel.py (v3) for the base design. v5 fuses the projection work into
the attention loop as paced "extras" chunks, moves all projection
epilogues and xT copies to DVE (ACT does exp only), and shares one PSUM
tag for all short-lived 2KB tiles.
"""

import numpy as np

import concourse.bass as bass
import concourse.mybir as mybir
from concourse import bacc
from concourse import tile
from concourse.bass_utils import run_bass_kernel_spmd
from concourse.masks import make_identity

F32 = mybir.dt.float32
F32R = mybir.dt.float32r

EMBED = 768
NHEAD = 12
DH = 64
B = 2
T = 4096
HPC = 3
CH = HPC * DH
NCORES = 8
QW = 512


def build_program(t=T):
    nqb = t // QW

    nc = bacc.Bacc("TRN2", target_bir_lowering=False, debug=False,
                   num_devices=NCORES)

    x_d = nc.dram_tensor("x", [t, EMBED], F32, kind="ExternalInput")
    wqkv_d = nc.dram_tensor("wqkvT", [EMBED, 576], F32, kind="ExternalInput")
    bqkv_d = nc.dram_tensor("bqkv", [576, 1], F32, kind="ExternalInput")
    wo_d = nc.dram_tensor("woT", [CH, EMBED], F32, kind="ExternalInput")
    y_d = nc.dram_tensor("y", [t, EMBED], F32, kind="ExternalOutput")

    Act = mybir.ActivationFunctionType

    with tile.TileContext(nc) as tc:
        with (
            tc.tile_pool(name="const", bufs=1) as cpool,
            tc.tile_pool(name="persist", bufs=1) as perm,
        ):
            ident = cpool.tile([128, 128], F32, tag="ident")
            make_identity(nc, ident)
            identr = cpool.tile([128, 128], F32R, tag="identr")
            nc.vector.tensor_copy(identr, ident)
            ones_t = cpool.tile([128, 64], F32R, tag="ones")
            nc.gpsimd.memset(ones_t.bitcast(F32), 1.0)

            wqkv_sb = [cpool.tile([128, 576], F32R, name=f"wqkv{kt}",
                                  tag=f"wqkv{kt}") for kt in range(6)]
            wo_sb = [cpool.tile([64, EMBED], F32R, name=f"wo{h}",
                                tag=f"wo{h}") for h in range(3)]
            bias_sb = []
            for mc in range(5):
                mw = 128 if mc < 4 else 64
                b_t = cpool.tile([128, 1], F32, name=f"bias{mc}",
                                 tag=f"bias{mc}")
                nc.sync.dma_start(b_t[:mw, :],
                                  bqkv_d[mc * 128:mc * 128 + mw, :])
                bias_sb.append(b_t)
            with tc.tile_pool(name="wraw", bufs=1) as wraw:
                for kt in range(6):
                    w_raw = wraw.tile([128, 576], F32, name=f"wqkvraw{kt}",
                                      tag=f"wqkvraw{kt}")
                    nc.sync.dma_start(w_raw,
                                      wqkv_d[kt * 128:(kt + 1) * 128, :])
                    nc.vector.tensor_copy(wqkv_sb[kt], w_raw)
                for h in range(3):
                    wo_raw = wraw.tile([64, EMBED], F32, name=f"woraw{h}",
                                       tag=f"woraw{h}")
                    nc.sync.dma_start(wo_raw, wo_d[h * 64:(h + 1) * 64, :])
                    nc.vector.tensor_copy(wo_sb[h], wo_raw)

            q01 = perm.tile([128, t], F32R, tag="q01")
            k01 = perm.tile([128, t], F32R, tag="k01")
            qv0 = perm.tile([128, t], F32R, tag="qv0")
            kv1 = perm.tile([128, t], F32R, tag="kv1")
            v2t = perm.tile([64, t], F32R, tag="v2t")
            vs = [perm.tile([128, (t // 128) * 65], F32R, name=f"vs{h}",
                            tag=f"vs{h}")
                  for h in range(3)]
            for h in range(3):
                nc.gpsimd.memset(vs[h].bitcast(F32), 1.0)

            proj_dest = [q01, k01, qv0, kv1, v2t]

            def q_ap(h):
                return (q01[0:64], q01[64:128], qv0[0:64])[h]

            def k_ap(h):
                return (k01[0:64], k01[64:128], kv1[0:64])[h]

            v_src = [qv0[64:128], kv1[64:128], v2t[0:64]]
            v_idn = [identr[64:128, 64:128], identr[64:128, 64:128],
                     identr[0:64, 0:64]]

            import contextlib
            stack = contextlib.ExitStack()
            xpool = stack.enter_context(tc.tile_pool(name="xpool", bufs=4))
            xtpool = stack.enter_context(tc.tile_pool(name="xtpool", bufs=2))
            spsum = stack.enter_context(
                tc.tile_pool(name="spsum", bufs=2, space="PSUM"))
            accpsum = stack.enter_context(
                tc.tile_pool(name="accpsum", bufs=1, space="PSUM"))
            upsum = stack.enter_context(
                tc.tile_pool(name="upsum", bufs=3, space="PSUM"))
            ppool = stack.enter_context(tc.tile_pool(name="ppool", bufs=3))
            apool = stack.enter_context(tc.tile_pool(name="apool", bufs=2))
            rpool = stack.enter_context(tc.tile_pool(name="rpool", bufs=2))
            ysb = stack.enter_context(tc.tile_pool(name="ysb", bufs=2))

            def a_chunks(tb):
                xns = []
                xts = []

                def c_load():
                    for i in range(4):
                        row0 = tb * QW + i * 128
                        xn = xpool.tile([128, EMBED], F32, tag="xn",
                                        name=f"xn{tb}_{i}")
                        nc.sync.dma_start(xn, x_d[row0:row0 + 128, :])
                        xns.append(xn)
                    for ct in range(6):
                        xts.append(xtpool.tile(
                            [128, 512], F32R, tag=f"xt{ct}",
                            name=f"xt{ct}_{tb}"))

                def c_tr(ct):
                    def f():
                        tpg = upsum.tile([128, 512], F32, tag="u2k",
                                         name=f"tpg{tb}_{ct}")
                        for i in range(4):
                            nc.tensor.transpose(
                                tpg[:, i * 128:(i + 1) * 128],
                                xns[i][:, ct * 128:(ct + 1) * 128], ident)
                        nc.vector.tensor_copy(xts[ct], tpg)
                    return f

                def c_proj(mc):
                    def f():
                        mw = 128 if mc < 4 else 64
                        ps = upsum.tile([128, 512], F32, tag="u2k",
                                        name=f"proj{tb}_{mc}")
                        for ct in range(6):
                            nc.tensor.matmul(
                                ps[:mw, :],
                                lhsT=wqkv_sb[ct][:, mc * 128:mc * 128 + mw],
                                rhs=xts[ct],
                                start=(ct == 0), stop=(ct == 5))
                        dest = proj_dest[mc][:, tb * QW:(tb + 1) * QW]
                        nc.vector.tensor_scalar_add(dest, ps[:mw, :],
                                                    bias_sb[mc][:mw, :])
                    return f

                def c_vt(h):
                    def f():
                        vtile = upsum.tile([128, 512], F32R, tag="u2k",
                                           name=f"vt{h}_{tb}")
                        for i in range(4):
                            ck = tb * 4 + i
                            nc.tensor.transpose(
                                vtile[:, i * 64:(i + 1) * 64],
                                v_src[h][:, ck * 128:(ck + 1) * 128],
                                v_idn[h])
                        src = vtile[:, 0:256].rearrange(
                            "p (c w) -> p c w", w=64)
                        dst = vs[h].rearrange("p (c w) -> p c w", w=65)[
                            :, tb * 4:tb * 4 + 4, 0:64]
                        nc.vector.tensor_copy(dst, src)
                    return f

                chunks = [c_load]
                chunks += [c_tr(ct) for ct in range(6)]
                chunks += [c_proj(mc) for mc in range(5)]
                chunks += [c_vt(h) for h in range(3)]
                return chunks

            stages = []
            for qb in range(nqb):
                kbn = (qb + 1) * QW // 128
                ng = kbn // 2
                for h in range(3):
                    for g in range(ng):
                        stages.append((qb, h, g, 2 * g, g == ng - 1))
            nstages = len(stages)

            sp_t = {}
            pt_t = {}
            acc_t = {}
            rec_t = {}
            attn = {}
            deferred = {}

            def defer(slot, fn):
                deferred.setdefault(slot, []).append(fn)

            def emit_S(i):
                qb, h, g, kb0, last = stages[i]
                sp = spsum.tile([128, 2 * QW], F32, tag="s",
                                name=f"s{qb}_{h}_{g}")
                sp_t[i] = sp
                q_sl = slice(qb * QW, (qb + 1) * QW)
                for j in range(2):
                    kbi = kb0 + j
                    nc.tensor.matmul(
                        sp[:, j * QW:(j + 1) * QW],
                        lhsT=k_ap(h)[:, kbi * 128:(kbi + 1) * 128],
                        rhs=q_ap(h)[:, q_sl],
                        start=True, stop=True)

            def emit_exp_mask(i):
                qb, h, g, kb0, last = stages[i]
                kbn = (qb + 1) * QW // 128
                pt = ppool.tile([128, 2 * QW], F32R, tag="p",
                                name=f"p{qb}_{h}_{g}")
                pt_t[i] = pt
                nc.scalar.activation(pt, sp_t[i], Act.Exp,
                                     bias=0.0, scale=0.125)
                for j in range(2):
                    kbi = kb0 + j
                    if kbi >= kbn - 4:
                        v = pt[:, j * QW:(j + 1) * QW]
                        nc.gpsimd.affine_select(
                            out=v, in_=v,
                            compare_op=mybir.AluOpType.is_ge,
                            fill=0.0, base=qb * QW - kbi * 128,
                            pattern=[[1, QW]], channel_multiplier=-1)

            def emit_PV(i):
                qb, h, g, kb0, last = stages[i]
                if g == 0:
                    acc_t[(qb, h)] = accpsum.tile(
                        [65, QW], F32, tag="acc", name=f"acc{qb}_{h}")
                acc = acc_t[(qb, h)]
                kbn = (qb + 1) * QW // 128
                pt = pt_t.pop(i)
                for j in range(2):
                    kbi = kb0 + j
                    nc.tensor.matmul(
                        acc,
                        lhsT=vs[h][:, kbi * 65:kbi * 65 + 65],
                        rhs=pt[:, j * QW:(j + 1) * QW],
                        start=(kbi == 0), stop=(kbi == kbn - 1))
                sp_t.pop(i)

            def emit_recip(qb, h):
                acc = acc_t.pop((qb, h))
                accsb = rpool.tile([65, QW], F32, tag="accsb",
                                   name=f"accsb{qb}_{h}")
                nc.vector.tensor_copy(accsb, acc)
                rec = rpool.tile([65, QW], F32R, tag="rec",
                                 name=f"rec{qb}_{h}")
                rec_t[(qb, h)] = (accsb, rec)
                with nc.allow_low_precision(reason="fp32r rounding"):
                    nc.vector.reciprocal(rec[64:65], accsb[64:65])

            def emit_div(qb, h):
                accsb, rec = rec_t.pop((qb, h))
                bc = upsum.tile([128, 512], F32, tag="u2k",
                                name=f"bc{qb}_{h}")
                nc.tensor.matmul(bc[0:64, :], lhsT=ones_t[64:65, :],
                                 rhs=rec[64:65, :],
                                 start=True, stop=True)
                if h == 0:
                    attn[qb] = [apool.tile([64, QW], F32R, tag=f"attn{hh}",
                                           name=f"attn{hh}_{qb}")
                                for hh in range(3)]
                nc.vector.tensor_mul(attn[qb][h], accsb[0:64, :],
                                     bc[0:64, :])

            def emit_outproj(qb, mt):
                at = attn[qb]
                t_sl = slice(mt * 128, (mt + 1) * 128)
                row0 = qb * QW + mt * 128
                ys = ysb.tile([128, EMBED], F32, tag="ys",
                              name=f"ys{qb}_{mt}")
                for nh in range(2):
                    n_sl = slice(nh * 384, (nh + 1) * 384)
                    yp = upsum.tile([128, 512], F32, tag="u2k",
                                    name=f"y{qb}_{mt}_{nh}")
                    for h in range(3):
                        nc.tensor.matmul(yp[:, 0:384], lhsT=at[h][:, t_sl],
                                         rhs=wo_sb[h][:, n_sl],
                                         start=(h == 0), stop=(h == 2))
                    nc.vector.tensor_copy(ys[:, n_sl], yp[:, 0:384])
                nc.sync.dma_start(y_d[row0:row0 + 128, :], ys)
                if mt == 3:
                    attn.pop(qb)

            extras = []
            for fn in a_chunks(0):
                fn()
            extras.extend(a_chunks(1))

            emit_S(0)
            for i in range(nstages):
                qb, h, g, kb0, last = stages[i]
                if i + 1 < nstages:
                    if stages[i + 1][0] != qb:
                        while extras:
                            extras.pop(0)()
                        if qb + 2 < nqb:
                            extras.extend(a_chunks(qb + 2))
                    emit_S(i + 1)
                emit_exp_mask(i)
                for fn in deferred.pop(i, ()):
                    fn()
                emit_PV(i)
                if last:
                    emit_recip(qb, h)
                    defer(i + 1, lambda qb=qb, h=h: emit_div(qb, h))
                    if h == 2:
                        # enqueue out-proj only after div(qb, h2) has been
                        # emitted (same slot, deferred list runs in order)
                        defer(i + 1, lambda qb=qb: extras.extend(
                            lambda qb=qb, mt=mt: emit_outproj(qb, mt)
                            for mt in range(4)))
                if extras:
                    extras.pop(0)()
            for slot in sorted(deferred):
                for fn in deferred[slot]:
                    fn()
            while extras:
                extras.pop(0)()
            stack.close()
    nc.compile()
    return nc


_PROG_CACHE = {}


def _get_program(t=T):
    if t not in _PROG_CACHE:
        _PROG_CACHE[t] = build_program(t)
    return _PROG_CACHE[t]


def make_in_maps(x, wq, bq, wk, bk, wv, bv, wo):
    in_maps = []
    for core in range(NCORES):
        b = core // 4
        hs = (core % 4) * HPC
        sl = [slice((hs + h) * DH, (hs + h + 1) * DH) for h in range(HPC)]
        cols = [wq[sl[0]].T, wq[sl[1]].T, wk[sl[0]].T, wk[sl[1]].T,
                wq[sl[2]].T, wv[sl[0]].T, wk[sl[2]].T, wv[sl[1]].T,
                wv[sl[2]].T]
        biases = [bq[sl[0]], bq[sl[1]], bk[sl[0]], bk[sl[1]],
                  bq[sl[2]], bv[sl[0]], bk[sl[2]], bv[sl[1]], bv[sl[2]]]
        wqkvT = np.ascontiguousarray(np.concatenate(cols, axis=1),
                                     dtype=np.float32)
        bqkv = np.ascontiguousarray(
            np.concatenate(biases)[:, None], dtype=np.float32)
        ch = slice(hs * DH, (hs + HPC) * DH)
        woT = np.ascontiguousarray(wo[:, ch].T, dtype=np.float32)
        in_maps.append({
            "x": np.ascontiguousarray(x[b], dtype=np.float32),
            "wqkvT": wqkvT,
            "bqkv": bqkv,
            "woT": woT,
        })
    return in_maps


def run(inputs, t=T, trace=False, **kw):
    arrs = {k: np.asarray(v, dtype=np.float32) for k, v in inputs.items()}
    nc = _get_program(t)
    in_maps = make_in_maps(**arrs)
    res = run_bass_kernel_spmd(nc, in_maps, list(range(NCORES)),
                               trace=trace, **kw)
    outs = [np.asarray(m["y"], dtype=np.float32) for m in res.results]
    y = np.empty((B, t, EMBED), dtype=np.float32)
    for b in range(B):
        y[b] = outs[4 * b] + outs[4 * b + 1] + outs[4 * b + 2] + outs[4 * b + 3]
    return y, res


def kernel(**inputs):
    y, _ = run(inputs)
    return y
